# revision 1
# baseline (speedup 1.0000x reference)
"""HarmonicNoiseSynth Trainium2 kernel.

Sharding: 8 cores = 4 batches x 2 harmonic halves (64 harmonics each).
Each core also handles 16 of the 32 noise bands; cores with j==0 compute the
modulator (noise-burst) path for harmonics 0..3. Host combines partials.

Per-core layout: harmonic rows split into 4 h-groups of 16; partitions hold
(h_local, tb) = h_local*8 + tb where tb indexes 8 time slices of 8192; free
dim is time within the slice, processed in 8 chunks of 1024.

Phase accumulation runs in Hz units (scan of masked frequencies) with
mod-48000 reductions at chunk boundaries, slice carries via a PE shift-matrix
matmul, and a final round-reduce; cos(x) = Sin(k*m + k*carry_term) with
k = 2*pi/48000 folded into the activation scale so the Sin argument stays in
[-pi, pi] where the LUT is valid. Per-time sums over harmonics/bands/
modulators are PE matmuls with block-diagonal indicator matrices (contraction
over partitions).
"""
import sys

sys.path.insert(0, "/opt/trn_rl_repo")

import numpy as np

import concourse.bass as bass
import concourse.mybir as mybir
from concourse.tile import TileContext
from concourse.bass_utils import run_bass_kernel_spmd

F = mybir.dt.float32
SR = 48000.0
K = float(2.0 * np.pi / SR)
INV_SR = float(1.0 / SR)
RC = float(1.5 * 2**23)  # fp32 round-to-nearest-integer magic constant
B, H, NB, T = 4, 128, 32, 65536
NTB = 8          # time slices on partitions
TS = T // NTB    # 8192 per slice
TC = 1024        # chunk columns
NCH = TS // TC   # 8 chunks
NG = 4           # h-groups of 16 harmonics
HG = 16
NM = 4           # modulators

_CACHE = {}


def _round_cols(nc, pool, out_col, in_col, modulus):
    """out = in - modulus*round(in/modulus) on a [128,1] column (3 tiny DVE ops)."""
    t1 = pool.tile([128, 1], F, tag="rc1")
    nc.vector.tensor_scalar(out=t1, in0=in_col, scalar1=float(1.0 / modulus),
                            scalar2=RC, op0=mybir.AluOpType.mult,
                            op1=mybir.AluOpType.add)
    t2 = pool.tile([128, 1], F, tag="rc2")
    nc.vector.tensor_scalar(out=t2, in0=t1, scalar1=RC, scalar2=float(-modulus),
                            op0=mybir.AluOpType.subtract,
                            op1=mybir.AluOpType.mult)
    # out = in + (-modulus * round) ; t2 = -modulus*round
    nc.vector.tensor_add(out=out_col, in0=in_col, in1=t2)


def _split_multiwaits(nc):
    """This walrus build supports ONE sync wait per instruction; hoist extras
    onto single-wait NoOps inserted before the offending instruction."""
    ctr = 0
    for f in nc.m.functions:
        for bb in f.blocks:
            insts = list(bb.instructions)
            if not any(i.sync_info is not None and len(i.sync_info.on_wait) > 1
                       for i in insts):
                continue
            new = []
            for inst in insts:
                si = inst.sync_info
                if si is not None and len(si.on_wait) > 1:
                    waits = list(si.on_wait)
                    for w in waits[:-1]:
                        ctr += 1
                        nop = mybir.InstNoOp(name=f"mwsplit_{ctr}",
                                             engine=inst.engine)
                        nop.sync_info = mybir.SyncInfo(on_wait=[w], on_update=[])
                        new.append(nop)
                    inst.sync_info = mybir.SyncInfo(on_wait=[waits[-1]],
                                                    on_update=list(si.on_update))
                new.append(inst)
            bb.instructions = new
    return ctr


def _build():
    nc = bass.Bass("TRN2")
    HN = H // 2  # 64 harmonics per core

    freq_d = nc.dram_tensor("freq", [HN, T], F, kind="ExternalInput")
    amp_d = nc.dram_tensor("amp", [HN, T], F, kind="ExternalInput")
    nba_d = nc.dram_tensor("nba", [NB // 2, T], F, kind="ExternalInput")
    nbb_d = nc.dram_tensor("nbb", [NB // 2, T], F, kind="ExternalInput")
    phiHz_d = nc.dram_tensor("phiHz", [128, NG], F, kind="ExternalInput")
    shiftM_d = nc.dram_tensor("shiftM", [128, 128], F, kind="ExternalInput")
    lhsT8_d = nc.dram_tensor("lhsT8", [128, 8], F, kind="ExternalInput")
    wlhsT_d = nc.dram_tensor("wlhsT", [128, 32], F, kind="ExternalInput")
    ecol_d = nc.dram_tensor("ecol", [128, 1], F, kind="ExternalInput")

    hc_d = nc.dram_tensor("hc_out", [8, TS], F, kind="ExternalOutput")
    nz_d = nc.dram_tensor("nz_out", [8, TS], F, kind="ExternalOutput")
    md_d = nc.dram_tensor("md_out", [2, 32, TC], F, kind="ExternalOutput")

    freq_r = freq_d[:, :].rearrange("h (tb t) -> (h tb) t", tb=NTB)   # [512, 8192]
    amp_r = amp_d[:, :].rearrange("h (tb t) -> (h tb) t", tb=NTB)
    nba_r = nba_d[:, :].rearrange("n (tb t) -> (n tb) t", tb=NTB)     # [128, 8192]
    nbb_r = nbb_d[:, :].rearrange("n (tb t) -> (n tb) t", tb=NTB)

    with TileContext(nc) as tc:
        with tc.tile_pool(name="big", bufs=1) as big, \
             tc.tile_pool(name="chunks", bufs=2) as ch, \
             tc.tile_pool(name="small", bufs=1) as sm, \
             tc.tile_pool(name="psum", bufs=2, space="PSUM") as pp:

            # constants
            lhsT8 = sm.tile([128, 8], F)
            nc.sync.dma_start(out=lhsT8, in_=lhsT8_d[:, :])
            shiftM = sm.tile([128, 128], F)
            nc.sync.dma_start(out=shiftM, in_=shiftM_d[:, :])
            phiHz = sm.tile([128, NG], F)
            nc.sync.dma_start(out=phiHz, in_=phiHz_d[:, :])
            wlhsT = sm.tile([128, 32], F)
            nc.sync.dma_start(out=wlhsT, in_=wlhsT_d[:, :])
            ecol = sm.tile([128, 1], F)
            nc.sync.dma_start(out=ecol, in_=ecol_d[:, :])
            zcol = sm.tile([128, 1], F)
            nc.vector.memset(zcol, 0.0)

            hc_sb = big.tile([8, TS], F)               # hc accumulator (pair sums)
            phase = [big.tile([128, TS], F, tag=f"phase{i}", name=f"phase{i}") for i in range(2)]
            staging = [big.tile([128, TC], F, tag=f"stage{i}", name=f"stage{i}") for i in range(2)]
            bias_sin = [sm.tile([128, 1], F, tag=f"bs{g}", name=f"bs{g}") for g in range(NG)]
            bias_qf = [sm.tile([128, 1], F, tag=f"bq{g}", name=f"bq{g}") for g in range(NG)]

            def l1(g):
                """mask+scan h-group g into phase[g % 2]; compute carry biases."""
                pb = phase[g % 2]
                prev_col = None
                for c in range(NCH):
                    ft = ch.tile([128, TC], F, tag="freq")
                    nc.sync.dma_start(
                        out=ft, in_=freq_r[g * 128:(g + 1) * 128,
                                           c * TC:(c + 1) * TC])
                    # masked f in-place: (f < SR/2) * f
                    nc.vector.scalar_tensor_tensor(
                        out=ft, in0=ft, scalar=float(SR / 2), in1=ft,
                        op0=mybir.AluOpType.is_lt, op1=mybir.AluOpType.mult)
                    seg = pb[:, c * TC:(c + 1) * TC]
                    nc.vector.tensor_tensor_scan(
                        out=seg, data0=ft, data1=ft,
                        initial=(zcol if prev_col is None else prev_col),
                        op0=mybir.AluOpType.add, op1=mybir.AluOpType.bypass)
                    red = sm.tile([128, 1], F, tag=f"red{c % 2}")
                    _round_cols(nc, sm, red, seg[:, TC - 1:TC], SR)
                    prev_col = red
                # slice carries: shiftM.T @ totals (totals = prev_col, reduced)
                cps = pp.tile([128, 1], F, tag="md_ps", bufs=1, name="cps")
                nc.tensor.matmul(cps, shiftM, prev_col, start=True, stop=True)
                csb = sm.tile([128, 1], F, tag="carry_sb")
                nc.scalar.copy(out=csb, in_=cps)
                cred = sm.tile([128, 1], F, tag="carry_red")
                _round_cols(nc, sm, cred, csb, SR)
                cb = sm.tile([128, 1], F, tag="cb")
                nc.vector.tensor_add(out=cb, in0=cred, in1=phiHz[:, g:g + 1])
                nc.vector.tensor_scalar(out=bias_sin[g], in0=cb, scalar1=K,
                                        scalar2=None, op0=mybir.AluOpType.mult)
                nc.vector.tensor_scalar(out=bias_qf[g], in0=cb, scalar1=INV_SR,
                                        scalar2=None, op0=mybir.AluOpType.mult)

            def l2_pair(pair_idx, gs):
                """consume phase bufs for groups gs (len 2); accumulate hc."""
                for c in range(NCH):
                    ps = pp.tile([8, TC], F, tag="hc_ps", bufs=2, name="ps")
                    for i, g in enumerate(gs):
                        pb = phase[g % 2]
                        seg = pb[:, c * TC:(c + 1) * TC]
                        qf = ch.tile([128, TC], F, tag="qf")
                        # qf = phase/SR + carry_term/SR
                        nc.scalar.activation(
                            out=qf, in_=seg,
                            func=mybir.ActivationFunctionType.Identity,
                            scale=INV_SR, bias=bias_qf[g])
                        # rnd = round(qf) in-place (Pool, 1-input)
                        nc.gpsimd.tensor_scalar(
                            out=qf, in0=qf, scalar1=RC, scalar2=RC,
                            op0=mybir.AluOpType.add,
                            op1=mybir.AluOpType.subtract)
                        # m = phase - SR*rnd  (in-place on qf)
                        nc.vector.scalar_tensor_tensor(
                            out=qf, in0=qf, scalar=-SR, in1=seg,
                            op0=mybir.AluOpType.mult, op1=mybir.AluOpType.add)
                        cosv = ch.tile([128, TC], F, tag="cos")
                        nc.scalar.activation(
                            out=cosv, in_=qf,
                            func=mybir.ActivationFunctionType.Sin,
                            scale=K, bias=bias_sin[g])
                        if g == 0:
                            half, cl = divmod(c, NCH // 2)
                            nc.sync.dma_start(
                                out=staging[half][cl * 32:(cl + 1) * 32, :],
                                in_=cosv[0:32, :])
                        at = ch.tile([128, TC], F, tag="amp")
                        nc.sync.dma_start(
                            out=at, in_=amp_r[g * 128:(g + 1) * 128,
                                              c * TC:(c + 1) * TC])
                        # prod in-place on cosv (Pool 2-input)
                        nc.gpsimd.tensor_mul(out=cosv, in0=cosv, in1=at)
                        for s in range(TC // 512):
                            nc.tensor.matmul(
                                ps[:, s * 512:(s + 1) * 512], lhsT8,
                                cosv[:, s * 512:(s + 1) * 512],
                                start=(i == 0), stop=(i == len(gs) - 1))
                    dst = hc_sb[:, c * TC:(c + 1) * TC]
                    if pair_idx == 0:
                        nc.scalar.copy(out=dst, in_=ps)
                    else:
                        nc.vector.tensor_add(out=dst, in0=dst, in1=ps)

            l1(0)
            l1(1)
            l2_pair(0, [0, 1])
            l1(2)
            l1(3)
            l2_pair(1, [2, 3])
            nc.sync.dma_start(out=hc_d[:, :], in_=hc_sb)

            # ---- noise path (g-independent) ----
            for c in range(NCH):
                bt = ch.tile([128, TC], F, tag="bands")
                nc.sync.dma_start(out=bt, in_=nbb_r[:, c * TC:(c + 1) * TC])
                atn = ch.tile([128, TC], F, tag="nba")
                nc.sync.dma_start(out=atn, in_=nba_r[:, c * TC:(c + 1) * TC])
                nc.vector.tensor_mul(out=bt, in0=bt, in1=atn)
                nps = pp.tile([8, TC], F, tag="nz_ps", bufs=1, name="nps")
                for s in range(TC // 512):
                    nc.tensor.matmul(nps[:, s * 512:(s + 1) * 512], lhsT8,
                                     bt[:, s * 512:(s + 1) * 512],
                                     start=True, stop=True)
                ncp = ch.tile([8, TC], F, tag="nz_sb")
                nc.scalar.copy(out=ncp, in_=nps)
                nc.sync.dma_start(out=nz_d[:, c * TC:(c + 1) * TC], in_=ncp)

            # ---- modulator path on staging tiles (harmonics 0..3) ----
            for half in range(2):
                st = staging[half]
                y = ch.tile([128, TC], F, tag="md_y")
                nc.scalar.mul(out=y, in_=st, mul=0.99)
                y2 = ch.tile([128, TC], F, tag="md_y2")
                nc.vector.tensor_mul(out=y2, in0=y, in1=y)
                nc.scalar.activation(out=y2, in_=y2,
                                     func=mybir.ActivationFunctionType.Sqrt,
                                     scale=-1.0, bias=1.0)
                nc.vector.reciprocal(out=y2, in_=y2)
                nc.vector.tensor_mul(out=y2, in0=y, in1=y2)
                nc.scalar.activation(out=y2, in_=y2,
                                     func=mybir.ActivationFunctionType.Arctan)
                nc.scalar.activation(out=y2, in_=y2,
                                     func=mybir.ActivationFunctionType.Abs,
                                     scale=float(2.0 / np.pi))
                nc.scalar.activation(out=y2, in_=y2,
                                     func=mybir.ActivationFunctionType.Ln)
                nc.vector.tensor_scalar_mul(out=y2, in0=y2, scalar1=ecol)
                nc.scalar.activation(out=y2, in_=y2,
                                     func=mybir.ActivationFunctionType.Exp)
                mps = pp.tile([32, TC], F, tag="md_ps", bufs=1, name="mps")
                for s in range(TC // 512):
                    nc.tensor.matmul(mps[:, s * 512:(s + 1) * 512], wlhsT,
                                     y2[:, s * 512:(s + 1) * 512],
                                     start=True, stop=True)
                mcp = ch.tile([32, TC], F, tag="md_sb")
                nc.scalar.copy(out=mcp, in_=mps)
                nc.sync.dma_start(out=md_d[half, :, :], in_=mcp)

    _split_multiwaits(nc)
    return nc


def kernel(**inputs):
    hf = np.ascontiguousarray(np.asarray(inputs["harmonic_frequencies"], np.float32))
    ha = np.ascontiguousarray(np.asarray(inputs["harmonic_amplitudes"], np.float32))
    nba = np.ascontiguousarray(np.asarray(inputs["noisebank_amplitudes"], np.float32))
    nbe = np.asarray(inputs["noisebank_mod_exponents"], np.float32)
    nbw = np.asarray(inputs["noisebank_mod_weights"], np.float32)
    pg = np.asarray(inputs["pulse_noise_gain"], np.float32)
    fg = np.asarray(inputs["flow_noise_gain"], np.float32)
    ip = np.asarray(inputs["initial_phase"], np.float32)
    nbands = np.ascontiguousarray(np.asarray(inputs["noise_bands"], np.float32))

    if "nc" not in _CACHE:
        _CACHE["nc"] = _build()
    nc = _CACHE["nc"]

    # host-side constant matrices (core-independent)
    p = np.arange(128)
    tb_p = p % 8
    lhsT8 = (tb_p[:, None] == np.arange(8)[None, :]).astype(np.float32)
    shiftM = ((p[:, None] // 8 == p[None, :] // 8) &
              (tb_p[:, None] < tb_p[None, :])).astype(np.float32)
    m_p = (p % 32) // 8           # modulator index per staging partition
    cl_p = p // 32                # chunk-local index per staging partition
    jj = np.arange(32)
    ind_mod = ((cl_p[:, None] == jj[None, :] // 8) &
               (tb_p[:, None] == jj[None, :] % 8)).astype(np.float32)

    in_maps = []
    for core in range(8):
        b, j = divmod(core, 2)
        hs = slice(j * 64, j * 64 + 64)
        ns = slice(j * 16, j * 16 + 16)
        # phiHz: (initial_phase + pi/2)/K per (h_local within group g, tb)
        iphz = ((ip[b, hs, 0].astype(np.float64) + np.pi / 2) / K).astype(np.float32)
        phiHz = np.zeros((128, NG), np.float32)
        for g in range(NG):
            phiHz[:, g] = np.repeat(iphz[g * HG:(g + 1) * HG], 8)
        wl = (ind_mod * nbw[b, m_p, 0][:, None]).astype(np.float32)
        ecol = nbe[b, m_p, 0].astype(np.float32).reshape(128, 1)
        in_maps.append(dict(
            freq=hf[b, hs], amp=ha[b, hs], nba=nba[b, ns], nbb=nbands[ns],
            phiHz=phiHz, shiftM=shiftM, lhsT8=lhsT8, wlhsT=wl, ecol=ecol))

    res = run_bass_kernel_spmd(nc, in_maps, core_ids=list(range(8)))
    outs = res.results

    # host combine
    out = np.empty((B, 1, T), np.float32)
    for b in range(B):
        r0, r1 = outs[2 * b], outs[2 * b + 1]
        hc = (r0["hc_out"].reshape(T) + r1["hc_out"].reshape(T))
        noise = (r0["nz_out"].reshape(T) + r1["nz_out"].reshape(T))
        # mod: md_out[half, j', tl], j' = c_local*8 + tb, t = tb*8192 + c*1024 + tl
        md = r0["md_out"].reshape(2, 4, 8, TC)          # [half, c_local, tb, tl]
        msum = np.ascontiguousarray(md.transpose(2, 0, 1, 3)).reshape(T)
        pgb = pg[b, 0, 0]; fgb = fg[b, 0, 0]
        tg = (pgb + fgb) * np.float32(0.7)
        out[b, 0] = (hc + msum * noise * pgb + hc * noise * tg
                     + noise * fgb * np.float32(0.3))
    return out



# revision 2
# speedup vs baseline: 1.2094x; 1.2094x over previous
"""HarmonicNoiseSynth Trainium2 kernel.

Sharding: 8 cores = 4 batches x 2 harmonic halves (64 harmonics each).
Each core also handles 16 of the 32 noise bands; cores with j==0 compute the
modulator (noise-burst) path for harmonics 0..3. Host combines partials.

Per-core layout: harmonic rows split into 4 h-groups of 16; partitions hold
(h_local, tb) = h_local*8 + tb where tb indexes 8 time slices of 8192; free
dim is time within the slice, processed in 8 chunks of 1024.

Phase accumulation runs in Hz units (scan of masked frequencies) with
mod-48000 reductions at chunk boundaries, slice carries via a PE shift-matrix
matmul, and a final round-reduce; cos(x) = Sin(k*m + k*carry_term) with
k = 2*pi/48000 folded into the activation scale so the Sin argument stays in
[-pi, pi] where the LUT is valid. Per-time sums over harmonics/bands/
modulators are PE matmuls with block-diagonal indicator matrices (contraction
over partitions).
"""
import sys

sys.path.insert(0, "/opt/trn_rl_repo")

import numpy as np

import concourse.bass as bass
import concourse.mybir as mybir
from concourse.tile import TileContext
from concourse.bass_utils import run_bass_kernel_spmd

F = mybir.dt.float32
SR = 48000.0
K = float(2.0 * np.pi / SR)
INV_SR = float(1.0 / SR)
RC = float(1.5 * 2**23)  # fp32 round-to-nearest-integer magic constant
B, H, NB, T = 4, 128, 32, 65536
NTB = 8          # time slices on partitions
TS = T // NTB    # 8192 per slice
TC = 1024        # chunk columns
NCH = TS // TC   # 8 chunks
NG = 4           # h-groups of 16 harmonics
HG = 16
NM = 4           # modulators

_CACHE = {}


def _round_cols(nc, pool, out_col, in_col, modulus):
    """out = in - modulus*round(in/modulus) on a [128,1] column (3 tiny DVE ops)."""
    t1 = pool.tile([128, 1], F, tag="rc1")
    nc.vector.tensor_scalar(out=t1, in0=in_col, scalar1=float(1.0 / modulus),
                            scalar2=RC, op0=mybir.AluOpType.mult,
                            op1=mybir.AluOpType.add)
    t2 = pool.tile([128, 1], F, tag="rc2")
    nc.vector.tensor_scalar(out=t2, in0=t1, scalar1=RC, scalar2=float(-modulus),
                            op0=mybir.AluOpType.subtract,
                            op1=mybir.AluOpType.mult)
    # out = in + (-modulus * round) ; t2 = -modulus*round
    nc.vector.tensor_add(out=out_col, in0=in_col, in1=t2)


def _split_multiwaits(nc):
    """This walrus build supports ONE sync wait per instruction; hoist extras
    onto single-wait NoOps inserted before the offending instruction."""
    ctr = 0
    for f in nc.m.functions:
        for bb in f.blocks:
            insts = list(bb.instructions)
            if not any(i.sync_info is not None and len(i.sync_info.on_wait) > 1
                       for i in insts):
                continue
            new = []
            for inst in insts:
                si = inst.sync_info
                if si is not None and len(si.on_wait) > 1:
                    waits = list(si.on_wait)
                    for w in waits[:-1]:
                        ctr += 1
                        nop = mybir.InstNoOp(name=f"mwsplit_{ctr}",
                                             engine=inst.engine)
                        nop.sync_info = mybir.SyncInfo(on_wait=[w], on_update=[])
                        new.append(nop)
                    inst.sync_info = mybir.SyncInfo(on_wait=[waits[-1]],
                                                    on_update=list(si.on_update))
                new.append(inst)
            bb.instructions = new
    return ctr


def _build():
    nc = bass.Bass("TRN2")
    HN = H // 2  # 64 harmonics per core

    freq_d = nc.dram_tensor("freq", [HN, T], F, kind="ExternalInput")
    amp_d = nc.dram_tensor("amp", [HN, T], F, kind="ExternalInput")
    nba_d = nc.dram_tensor("nba", [NB // 2, T], F, kind="ExternalInput")
    nbb_d = nc.dram_tensor("nbb", [NB // 2, T], F, kind="ExternalInput")
    phiHz_d = nc.dram_tensor("phiHz", [128, NG], F, kind="ExternalInput")
    shiftM_d = nc.dram_tensor("shiftM", [128, 128], F, kind="ExternalInput")
    lhsT8_d = nc.dram_tensor("lhsT8", [128, 8], F, kind="ExternalInput")
    wlhsT_d = nc.dram_tensor("wlhsT", [128, 32], F, kind="ExternalInput")
    ecol_d = nc.dram_tensor("ecol", [128, 1], F, kind="ExternalInput")

    hc_d = nc.dram_tensor("hc_out", [8, TS], F, kind="ExternalOutput")
    nz_d = nc.dram_tensor("nz_out", [8, TS], F, kind="ExternalOutput")
    md_d = nc.dram_tensor("md_out", [2, 32, TC], F, kind="ExternalOutput")

    freq_r = freq_d[:, :].rearrange("h (tb t) -> (h tb) t", tb=NTB)   # [512, 8192]
    amp_r = amp_d[:, :].rearrange("h (tb t) -> (h tb) t", tb=NTB)
    nba_r = nba_d[:, :].rearrange("n (tb t) -> (n tb) t", tb=NTB)     # [128, 8192]
    nbb_r = nbb_d[:, :].rearrange("n (tb t) -> (n tb) t", tb=NTB)

    with TileContext(nc) as tc:
        with tc.tile_pool(name="big", bufs=1) as big, \
             tc.tile_pool(name="chunks", bufs=2) as ch, \
             tc.tile_pool(name="small", bufs=1) as sm, \
             tc.tile_pool(name="psum", bufs=2, space="PSUM") as pp:

            # constants
            lhsT8 = sm.tile([128, 8], F)
            nc.sync.dma_start(out=lhsT8, in_=lhsT8_d[:, :])
            shiftM = sm.tile([128, 128], F)
            nc.sync.dma_start(out=shiftM, in_=shiftM_d[:, :])
            phiHz = sm.tile([128, NG], F)
            nc.sync.dma_start(out=phiHz, in_=phiHz_d[:, :])
            wlhsT = sm.tile([128, 32], F)
            nc.sync.dma_start(out=wlhsT, in_=wlhsT_d[:, :])
            ecol = sm.tile([128, 1], F)
            nc.sync.dma_start(out=ecol, in_=ecol_d[:, :])
            zcol = sm.tile([128, 1], F)
            nc.vector.memset(zcol, 0.0)

            hc_sb = big.tile([8, TS], F)               # hc accumulator (pair sums)
            phase = [big.tile([128, TS], F, tag=f"phase{i}", name=f"phase{i}") for i in range(2)]
            staging = [big.tile([128, TC], F, tag=f"stage{i}", name=f"stage{i}") for i in range(2)]
            bias_sin = [sm.tile([128, 1], F, tag=f"bs{g}", name=f"bs{g}") for g in range(NG)]
            bias_qf = [sm.tile([128, 1], F, tag=f"bq{g}", name=f"bq{g}") for g in range(NG)]

            def l1(g):
                """mask+scan h-group g into phase[g % 2]; compute carry biases."""
                pb = phase[g % 2]
                prev_col = None
                for c in range(NCH):
                    ft = ch.tile([128, TC], F, tag="freq")
                    nc.sync.dma_start(
                        out=ft, in_=freq_r[g * 128:(g + 1) * 128,
                                           c * TC:(c + 1) * TC])
                    # masked f in-place: (f < SR/2) * f
                    nc.vector.scalar_tensor_tensor(
                        out=ft, in0=ft, scalar=float(SR / 2), in1=ft,
                        op0=mybir.AluOpType.is_lt, op1=mybir.AluOpType.mult)
                    seg = pb[:, c * TC:(c + 1) * TC]
                    nc.vector.tensor_tensor_scan(
                        out=seg, data0=ft, data1=ft,
                        initial=(zcol if prev_col is None else prev_col),
                        op0=mybir.AluOpType.add, op1=mybir.AluOpType.bypass)
                    red = sm.tile([128, 1], F, tag=f"red{c % 2}")
                    _round_cols(nc, sm, red, seg[:, TC - 1:TC], SR)
                    prev_col = red
                # slice carries: shiftM.T @ totals (totals = prev_col, reduced)
                cps = pp.tile([128, 1], F, tag="md_ps", bufs=1, name="cps")
                nc.tensor.matmul(cps, shiftM, prev_col, start=True, stop=True)
                csb = sm.tile([128, 1], F, tag="carry_sb")
                nc.scalar.copy(out=csb, in_=cps)
                cred = sm.tile([128, 1], F, tag="carry_red")
                _round_cols(nc, sm, cred, csb, SR)
                cb = sm.tile([128, 1], F, tag="cb")
                nc.vector.tensor_add(out=cb, in0=cred, in1=phiHz[:, g:g + 1])
                nc.vector.tensor_scalar(out=bias_sin[g], in0=cb, scalar1=K,
                                        scalar2=None, op0=mybir.AluOpType.mult)
                nc.vector.tensor_scalar(out=bias_qf[g], in0=cb, scalar1=INV_SR,
                                        scalar2=None, op0=mybir.AluOpType.mult)

            def l2_pair(pair_idx, gs):
                """consume phase bufs for groups gs (len 2); accumulate hc."""
                for c in range(NCH):
                    ps = pp.tile([8, TC], F, tag="hc_ps", bufs=2, name="ps")
                    for i, g in enumerate(gs):
                        pb = phase[g % 2]
                        seg = pb[:, c * TC:(c + 1) * TC]
                        qf = ch.tile([128, TC], F, tag="qf")
                        # qf = phase/SR + carry_term/SR
                        nc.scalar.activation(
                            out=qf, in_=seg,
                            func=mybir.ActivationFunctionType.Identity,
                            scale=INV_SR, bias=bias_qf[g])
                        # rnd = round(qf) in-place (Pool, 1-input)
                        nc.gpsimd.tensor_scalar(
                            out=qf, in0=qf, scalar1=RC, scalar2=RC,
                            op0=mybir.AluOpType.add,
                            op1=mybir.AluOpType.subtract)
                        # m = phase - SR*rnd  (in-place on qf)
                        nc.vector.scalar_tensor_tensor(
                            out=qf, in0=qf, scalar=-SR, in1=seg,
                            op0=mybir.AluOpType.mult, op1=mybir.AluOpType.add)
                        cosv = ch.tile([128, TC], F, tag="cos")
                        nc.scalar.activation(
                            out=cosv, in_=qf,
                            func=mybir.ActivationFunctionType.Sin,
                            scale=K, bias=bias_sin[g])
                        if g == 0:
                            half, cl = divmod(c, NCH // 2)
                            nc.sync.dma_start(
                                out=staging[half][cl * 32:(cl + 1) * 32, :],
                                in_=cosv[0:32, :])
                        at = ch.tile([128, TC], F, tag="amp")
                        nc.sync.dma_start(
                            out=at, in_=amp_r[g * 128:(g + 1) * 128,
                                              c * TC:(c + 1) * TC])
                        # prod in-place on cosv (Pool 2-input)
                        nc.gpsimd.tensor_mul(out=cosv, in0=cosv, in1=at)
                        for s in range(TC // 512):
                            nc.tensor.matmul(
                                ps[:, s * 512:(s + 1) * 512], lhsT8,
                                cosv[:, s * 512:(s + 1) * 512],
                                start=(i == 0), stop=(i == len(gs) - 1))
                    dst = hc_sb[:, c * TC:(c + 1) * TC]
                    if pair_idx == 0:
                        nc.scalar.copy(out=dst, in_=ps)
                    else:
                        nc.vector.tensor_add(out=dst, in0=dst, in1=ps)

            l1(0)
            l1(1)
            l2_pair(0, [0, 1])
            l1(2)
            l1(3)
            l2_pair(1, [2, 3])
            nc.sync.dma_start(out=hc_d[:, :], in_=hc_sb)

            # ---- noise path (g-independent) ----
            for c in range(NCH):
                bt = ch.tile([128, TC], F, tag="bands")
                nc.sync.dma_start(out=bt, in_=nbb_r[:, c * TC:(c + 1) * TC])
                atn = ch.tile([128, TC], F, tag="nba")
                nc.sync.dma_start(out=atn, in_=nba_r[:, c * TC:(c + 1) * TC])
                nc.vector.tensor_mul(out=bt, in0=bt, in1=atn)
                nps = pp.tile([8, TC], F, tag="nz_ps", bufs=1, name="nps")
                for s in range(TC // 512):
                    nc.tensor.matmul(nps[:, s * 512:(s + 1) * 512], lhsT8,
                                     bt[:, s * 512:(s + 1) * 512],
                                     start=True, stop=True)
                ncp = ch.tile([8, TC], F, tag="nz_sb")
                nc.scalar.copy(out=ncp, in_=nps)
                nc.sync.dma_start(out=nz_d[:, c * TC:(c + 1) * TC], in_=ncp)

            # ---- modulator path on staging tiles (harmonics 0..3) ----
            for half in range(2):
                st = staging[half]
                y = ch.tile([128, TC], F, tag="md_y")
                nc.scalar.mul(out=y, in_=st, mul=0.99)
                y2 = ch.tile([128, TC], F, tag="md_y2")
                nc.vector.tensor_mul(out=y2, in0=y, in1=y)
                nc.scalar.activation(out=y2, in_=y2,
                                     func=mybir.ActivationFunctionType.Sqrt,
                                     scale=-1.0, bias=1.0)
                nc.vector.reciprocal(out=y2, in_=y2)
                nc.vector.tensor_mul(out=y2, in0=y, in1=y2)
                nc.scalar.activation(out=y2, in_=y2,
                                     func=mybir.ActivationFunctionType.Arctan)
                nc.scalar.activation(out=y2, in_=y2,
                                     func=mybir.ActivationFunctionType.Abs,
                                     scale=float(2.0 / np.pi))
                nc.scalar.activation(out=y2, in_=y2,
                                     func=mybir.ActivationFunctionType.Ln)
                nc.vector.tensor_scalar_mul(out=y2, in0=y2, scalar1=ecol)
                nc.scalar.activation(out=y2, in_=y2,
                                     func=mybir.ActivationFunctionType.Exp)
                mps = pp.tile([32, TC], F, tag="md_ps", bufs=1, name="mps")
                for s in range(TC // 512):
                    nc.tensor.matmul(mps[:, s * 512:(s + 1) * 512], wlhsT,
                                     y2[:, s * 512:(s + 1) * 512],
                                     start=True, stop=True)
                mcp = ch.tile([32, TC], F, tag="md_sb")
                nc.scalar.copy(out=mcp, in_=mps)
                nc.sync.dma_start(out=md_d[half, :, :], in_=mcp)

    _split_multiwaits(nc)
    return nc


def kernel(**inputs):
    hf = np.ascontiguousarray(np.asarray(inputs["harmonic_frequencies"], np.float32))
    ha = np.ascontiguousarray(np.asarray(inputs["harmonic_amplitudes"], np.float32))
    nba = np.ascontiguousarray(np.asarray(inputs["noisebank_amplitudes"], np.float32))
    nbe = np.asarray(inputs["noisebank_mod_exponents"], np.float32)
    nbw = np.asarray(inputs["noisebank_mod_weights"], np.float32)
    pg = np.asarray(inputs["pulse_noise_gain"], np.float32)
    fg = np.asarray(inputs["flow_noise_gain"], np.float32)
    ip = np.asarray(inputs["initial_phase"], np.float32)
    nbands = np.ascontiguousarray(np.asarray(inputs["noise_bands"], np.float32))

    if "nc" not in _CACHE:
        _CACHE["nc"] = _build()
    nc = _CACHE["nc"]

    # host-side constant matrices (core-independent)
    p = np.arange(128)
    tb_p = p % 8
    lhsT8 = (tb_p[:, None] == np.arange(8)[None, :]).astype(np.float32)
    shiftM = ((p[:, None] // 8 == p[None, :] // 8) &
              (tb_p[:, None] < tb_p[None, :])).astype(np.float32)
    m_p = (p % 32) // 8           # modulator index per staging partition
    cl_p = p // 32                # chunk-local index per staging partition
    jj = np.arange(32)
    ind_mod = ((cl_p[:, None] == jj[None, :] // 8) &
               (tb_p[:, None] == jj[None, :] % 8)).astype(np.float32)

    in_maps = []
    for core in range(8):
        b, j = divmod(core, 2)
        hs = slice(j * 64, j * 64 + 64)
        ns = slice(j * 16, j * 16 + 16)
        # phiHz: (initial_phase + pi/2)/K per (h_local within group g, tb)
        iphz = ((ip[b, hs, 0].astype(np.float64) + np.pi / 2) / K).astype(np.float32)
        phiHz = np.zeros((128, NG), np.float32)
        for g in range(NG):
            phiHz[:, g] = np.repeat(iphz[g * HG:(g + 1) * HG], 8)
        wl = (ind_mod * nbw[b, m_p, 0][:, None]).astype(np.float32)
        ecol = nbe[b, m_p, 0].astype(np.float32).reshape(128, 1)
        in_maps.append(dict(
            freq=hf[b, hs], amp=ha[b, hs], nba=nba[b, ns], nbb=nbands[ns],
            phiHz=phiHz, shiftM=shiftM, lhsT8=lhsT8, wlhsT=wl, ecol=ecol))

    import os as _os
    _tr = bool(_os.environ.get("HNS_TRACE"))
    res = run_bass_kernel_spmd(
        nc, in_maps, core_ids=list(range(8)), trace=_tr,
        tmpdir=_os.environ.get("HNS_TRACE_DIR") or None)
    _CACHE["last_res"] = res
    outs = res.results

    # host combine
    out = np.empty((B, 1, T), np.float32)
    for b in range(B):
        r0, r1 = outs[2 * b], outs[2 * b + 1]
        hc = (r0["hc_out"].reshape(T) + r1["hc_out"].reshape(T))
        noise = (r0["nz_out"].reshape(T) + r1["nz_out"].reshape(T))
        # mod: md_out[half, j', tl], j' = c_local*8 + tb, t = tb*8192 + c*1024 + tl
        md = r0["md_out"].reshape(2, 4, 8, TC)          # [half, c_local, tb, tl]
        msum = np.ascontiguousarray(md.transpose(2, 0, 1, 3)).reshape(T)
        pgb = pg[b, 0, 0]; fgb = fg[b, 0, 0]
        tg = (pgb + fgb) * np.float32(0.7)
        out[b, 0] = (hc + msum * noise * pgb + hc * noise * tg
                     + noise * fgb * np.float32(0.3))
    return out



# revision 3
# speedup vs baseline: 1.2511x; 1.0345x over previous
"""HarmonicNoiseSynth Trainium2 kernel, v2.

Sharding: 8 cores = 4 batches x 2 time-halves (32768 samples each); every
core sees all 128 harmonics and all 32 noise bands for its sample range, so
the whole module (oscillator bank, noise bank, modulator pulses, output mix)
is computed per core with no partials to combine on the host.

Phase runs in *turns* (cycles): the host pre-scales frequencies by 1/48000
and supplies per-harmonic initial phases (plus, for the second half, the
first half's frequency sum) mod 1. On device each 1024-sample chunk is a DVE
scan chained through frac()-reduced carry columns; range reduction to
[-1/2, 1/2] is one Pool add-magic-constant op plus one fused DVE
(subtract, subtract) op, and cos(phase) = Sin(-2pi * negm) directly.

Per-time reductions are PE matmuls in float32r. Chunk sums land in psum
[32, 512] banks accumulated across all 32 chunks via a sliding-window
ones-column lhsT (each matmul writes one disjoint psum row), so each
reduction needs only one psum->sbuf copy at the end. The final output mix
(harmonics + noise bursts + turbulence + flow noise) is computed on device;
the host only reshapes per-core outputs into [B, 1, T].
"""
import os
import sys

sys.path.insert(0, "/opt/trn_rl_repo")

import numpy as np

import concourse.bass as bass
import concourse.mybir as mybir
from concourse.tile import TileContext
from concourse.bass_utils import run_bass_kernel_spmd

F = mybir.dt.float32
FR = mybir.dt.float32r
SR = 48000.0
TWO_PI = float(2.0 * np.pi)
RC = float(1.5 * 2**23)   # fp32 round-to-nearest-integer magic constant
B, H, NB, T = 4, 128, 32, 65536
TH = T // 2               # samples per core
TC = 1024                 # chunk (psum row) length
NCH = TH // TC            # 32 chunks
TD = 2048                 # DMA tile columns
ND = TH // TD             # 16 DMA tiles for freq/amp
NM = 4                    # modulators

_CACHE = {}


def _split_multiwaits(nc):
    """This walrus build supports ONE sync wait per instruction; hoist extras
    onto single-wait NoOps inserted before the offending instruction."""
    ctr = 0
    for f in nc.m.functions:
        for bb in f.blocks:
            insts = list(bb.instructions)
            if not any(i.sync_info is not None and len(i.sync_info.on_wait) > 1
                       for i in insts):
                continue
            new = []
            for inst in insts:
                si = inst.sync_info
                if si is not None and len(si.on_wait) > 1:
                    waits = list(si.on_wait)
                    for w in waits[:-1]:
                        ctr += 1
                        nop = mybir.InstNoOp(name=f"mwsplit_{ctr}",
                                             engine=inst.engine)
                        nop.sync_info = mybir.SyncInfo(on_wait=[w], on_update=[])
                        new.append(nop)
                    inst.sync_info = mybir.SyncInfo(on_wait=[waits[-1]],
                                                    on_update=list(si.on_update))
                new.append(inst)
            bb.instructions = new
    return ctr


def _build():
    nc = bass.Bass("TRN2")

    f_d = nc.dram_tensor("f", [H, TH], F, kind="ExternalInput")       # freq, turns
    fm_d = nc.dram_tensor("fm", [128, TC], F, kind="ExternalInput")   # mod freqs
    a_d = nc.dram_tensor("a", [H, TH], F, kind="ExternalInput")
    # host pre-packed: [s2][p = n*4 + c][t], chunk = 8*s2 + 2*c + t//1024
    nba_d = nc.dram_tensor("nba", [4, 128, TD], F, kind="ExternalInput")
    nbb_d = nc.dram_tensor("nbb", [4, 128, TD], F, kind="ExternalInput")
    init_d = nc.dram_tensor("init", [128, 1], F, kind="ExternalInput")
    hcb_d = nc.dram_tensor("hcb", [128, 63], FR, kind="ExternalInput")
    nzb_d = nc.dram_tensor("nzb", [128, 51], FR, kind="ExternalInput")
    mlh_d = nc.dram_tensor("mlh", [128, 32], FR, kind="ExternalInput")
    ecol_d = nc.dram_tensor("ecol", [128, 1], F, kind="ExternalInput")
    tg_d = nc.dram_tensor("tg", [128, 1], F, kind="ExternalInput")
    fg3_d = nc.dram_tensor("fg3", [128, 1], F, kind="ExternalInput")

    out_d = nc.dram_tensor("out", [NCH, TC], F, kind="ExternalOutput")

    # Bank layout: psum/mix row o = 2*(c%16) + u, column r in [0,512);
    # group g = c//16 selects the sbuf column half 512*g + r.
    # Sample t of chunk c: u = (t%1024)//512, r = t%512.
    with TileContext(nc) as tc:
        with tc.tile_pool(name="dma", bufs=4) as dp, \
             tc.tile_pool(name="work", bufs=2) as wp, \
             tc.tile_pool(name="small", bufs=1) as sm, \
             tc.tile_pool(name="psum", bufs=1, space="PSUM") as pp:

            # ---- first scan inputs before anything else ----
            init = sm.tile([128, 1], F)
            nc.sync.dma_start(out=init, in_=init_d[:, :])
            fdma = dp.tile([128, TC], F, tag="fdma", bufs=6)
            nc.sync.dma_start(out=fdma, in_=f_d[:, 0:TC])
            adma = dp.tile([128, TC], F, tag="adma", bufs=6)
            nc.sync.dma_start(out=adma, in_=a_d[:, 0:TC])

            # ---- constants ----
            hcb = sm.tile([128, 63], FR)
            nc.sync.dma_start(out=hcb, in_=hcb_d[:, :])
            nzb = sm.tile([128, 51], FR)
            nc.sync.dma_start(out=nzb, in_=nzb_d[:, :])
            mlh = sm.tile([128, 32], FR)
            nc.sync.dma_start(out=mlh, in_=mlh_d[:, :])
            ecol = sm.tile([128, 1], F)
            nc.sync.dma_start(out=ecol, in_=ecol_d[:, :])
            tg = sm.tile([128, 1], F)
            nc.sync.dma_start(out=tg, in_=tg_d[:, :])
            fg3 = sm.tile([128, 1], F)
            nc.sync.dma_start(out=fg3, in_=fg3_d[:, :])

            rc_col = sm.tile([128, 1], F)
            nc.vector.memset(rc_col, RC)
            # staging row p = 8*(c%16) + 2h + u, col = 512*(c//16) + r
            staging = sm.tile([128, TC], F)
            hc_ps = [pp.tile([NCH, 512], F, tag=f"hc{i}", bufs=1,
                             name=f"hc_ps{i}") for i in range(2)]
            nz_ps = [pp.tile([NCH, 512], F, tag=f"nz{i}", bufs=1,
                             name=f"nz_ps{i}") for i in range(2)]
            md_ps = [pp.tile([NCH, 512], F, tag=f"md{i}", bufs=1,
                             name=f"md_ps{i}") for i in range(2)]
            hc_sb = sm.tile([NCH, TC], F, tag="hc_sb")
            nz_sb = sm.tile([NCH, TC], F, tag="nz_sb")
            a_sb = sm.tile([NCH, TC], F, tag="a_sb")
            out_sb = sm.tile([NCH, TC], F, tag="out_sb")

            def noise_dma(s2):
                nbt = dp.tile([128, TD], F, tag="nbt", bufs=1)
                nc.sync.dma_start(out=nbt, in_=nba_d[s2, :, :])
                bbt = dp.tile([128, TD], F, tag="bbt", bufs=1)
                nc.sync.dma_start(out=bbt, in_=nbb_d[s2, :, :])
                return nbt, bbt

            def noise_compute(s2, nbt, bbt):
                # tile row p = 4n + ct covers chunk c = 8*s2 + 2*ct + k//2;
                # o = 16*(s2%2) + k + 4*(p%4), bank g = s2//2
                nt = wp.tile([128, TD], FR, tag="nt", bufs=1)
                nc.gpsimd.tensor_mul(out=nt, in0=nbt, in1=bbt)
                g = s2 // 2
                for k in range(4):
                    off = 16 * (s2 % 2) + k
                    nc.tensor.matmul(nz_ps[g], nzb[:, 19 - off:51 - off],
                                     nt[:, k * 512:(k + 1) * 512],
                                     start=(s2 % 2 == 0 and k == 0),
                                     stop=(s2 % 2 == 1 and k == 3),
                                     skip_group_check=True)

            def stage2(c, y_in):
                """negm -> sin -> staging -> amp product -> hc matmuls."""
                t1, P, ats = y_in
                g = c // 16
                negm = wp.tile([128, TC], F, tag="negm")
                nc.vector.scalar_tensor_tensor(
                    out=negm, in0=t1, scalar=RC, in1=P,
                    op0=mybir.AluOpType.subtract,
                    op1=mybir.AluOpType.subtract)
                y = wp.tile([128, TC], F, tag="y", bufs=3)
                nc.scalar.activation(out=y, in_=negm,
                                     func=mybir.ActivationFunctionType.Sin,
                                     scale=-TWO_PI)
                # issue from Act queue: keeps the SP load queue free of
                # compute-dependent DMAs (head-of-line blocking)
                nc.scalar.dma_start(
                    out=staging[8 * (c % 16):8 * (c % 16) + 8,
                                512 * g:512 * g + 512],
                    in_=y[0:4, :])
                prod = wp.tile([128, TC], FR, tag="prod", bufs=3)
                nc.gpsimd.tensor_mul(out=prod, in0=ats, in1=y)
                for u in range(2):
                    o = 2 * (c % 16) + u
                    nc.tensor.matmul(hc_ps[g], hcb[:, 31 - o:63 - o],
                                     prod[:, u * 512:(u + 1) * 512],
                                     start=(c % 16 == 0), stop=(c % 16 == 15),
                                     skip_group_check=True)

            def mod_path(g):
                """noise-burst shaping for column group g on staging, as a
                list of thunks (one op each) so group 0 can be drip-fed
                into the loop without bursting any engine."""
                st = staging[:, 512 * g:512 * g + 512]
                y2 = wp.tile([128, 512], F, tag="mchain", bufs=1)
                g2 = wp.tile([128, 512], F, tag="mchain2", bufs=1)
                g9 = wp.tile([128, 512], FR, tag="mchain3", bufs=1)
                A = mybir.ActivationFunctionType
                return [
                    lambda: nc.scalar.activation(out=y2, in_=st, func=A.Square),
                    lambda: nc.scalar.activation(out=g2, in_=y2, func=A.Sqrt,
                                                 scale=float(-0.99 * 0.99),
                                                 bias=1.0),
                    lambda: nc.vector.reciprocal(out=g2, in_=g2),
                    lambda: nc.vector.tensor_mul(out=g2, in0=g2, in1=st),
                    lambda: nc.scalar.activation(out=g2, in_=g2, func=A.Arctan,
                                                 scale=0.99),
                    lambda: nc.scalar.activation(out=g2, in_=g2, func=A.Abs,
                                                 scale=float(2.0 / np.pi)),
                    lambda: nc.scalar.activation(out=g2, in_=g2, func=A.Ln),
                    lambda: nc.vector.tensor_scalar_mul(out=g2, in0=g2,
                                                        scalar1=ecol),
                    lambda: nc.scalar.activation(out=g9, in_=g2, func=A.Exp),
                    lambda: nc.tensor.matmul(md_ps[g], mlh, g9, start=True,
                                             stop=True, skip_group_check=True),
                ]

            def finish_group(g):
                """psum -> sbuf, mix, store thunks for column group g."""
                cols = slice(512 * g, 512 * g + 512)
                A = mybir.ActivationFunctionType
                return [
                    lambda: nc.scalar.copy(out=hc_sb[:, cols], in_=hc_ps[g]),
                    lambda: nc.scalar.copy(out=nz_sb[:, cols], in_=nz_ps[g]),
                    # a = msum*pg + 0.3*fg   (pg folded into mlh weights)
                    lambda: nc.scalar.activation(out=a_sb[:, cols],
                                                 in_=md_ps[g], func=A.Identity,
                                                 scale=1.0, bias=fg3[0:NCH]),
                    # out = hc + noise * (a + hc*tg)
                    lambda: nc.vector.scalar_tensor_tensor(
                        out=out_sb[:, cols], in0=hc_sb[:, cols],
                        scalar=tg[0:NCH], in1=a_sb[:, cols],
                        op0=mybir.AluOpType.mult, op1=mybir.AluOpType.add),
                    lambda: nc.vector.tensor_mul(out=out_sb[:, cols],
                                                 in0=out_sb[:, cols],
                                                 in1=nz_sb[:, cols]),
                    lambda: nc.vector.tensor_add(out=out_sb[:, cols],
                                                 in0=out_sb[:, cols],
                                                 in1=hc_sb[:, cols]),
                    # dram rows 16g..16g+16: (crow, u, r) matches sbuf (o, r)
                    lambda: nc.sync.dma_start(out=out_d[16 * g:16 * g + 16, :],
                                              in_=out_sb[:, cols]),
                ]

            # ---- harmonic loop (2-stage software pipeline skew) ----
            carry = None
            pending = []
            noise_tiles = None
            deferred = []         # drip-fed group-0 tail work
            for c in range(NCH):
                if c > 0:
                    fdma = dp.tile([128, TC], F, tag="fdma", bufs=6)
                    nc.sync.dma_start(out=fdma, in_=f_d[:, c * TC:(c + 1) * TC])
                    adma = dp.tile([128, TC], F, tag="adma", bufs=6)
                    nc.sync.dma_start(out=adma, in_=a_d[:, c * TC:(c + 1) * TC])
                if c % 8 == 1:
                    noise_tiles = noise_dma(c // 8)
                fts = fdma[:, 0:TC]
                ats = adma[:, 0:TC]

                P = wp.tile([128, TC], F, tag="P", bufs=4)
                nc.vector.tensor_tensor_scan(
                    out=P, data0=fts, data1=fts,
                    initial=(init if c == 0 else carry),
                    op0=mybir.AluOpType.add, op1=mybir.AluOpType.bypass)
                if c + 1 < NCH:
                    rcol = sm.tile([128, 1], F, tag=f"rcol{c % 2}")
                    nc.vector.tensor_scalar(
                        out=rcol, in0=P[:, TC - 1:TC], scalar1=RC, scalar2=RC,
                        op0=mybir.AluOpType.add, op1=mybir.AluOpType.subtract)
                    ncar = sm.tile([128, 1], F, tag=f"carry{c % 2}")
                    nc.vector.tensor_tensor(out=ncar, in0=P[:, TC - 1:TC],
                                            in1=rcol,
                                            op=mybir.AluOpType.subtract)
                    carry = ncar
                t1 = wp.tile([128, TC], F, tag="t1", bufs=4)
                nc.scalar.activation(out=t1, in_=P,
                                     func=mybir.ActivationFunctionType.Identity,
                                     scale=1.0, bias=rc_col)

                pending.append((c, t1, P, ats))
                if len(pending) > 2:
                    cc, *rest = pending.pop(0)
                    stage2(cc, rest)
                if c % 8 == 3:
                    noise_compute(c // 8, *noise_tiles)
                if c == 18:
                    # group 0 fully staged (chunks 0..15 done by c=17)
                    deferred = mod_path(0) + finish_group(0)
                if deferred:
                    deferred.pop(0)()
            for th in deferred:
                th()
            for cc, *rest in pending:
                stage2(cc, rest)
            for th in mod_path(1) + finish_group(1):
                th()

    _split_multiwaits(nc)
    return nc


def kernel(**inputs):
    hf = np.asarray(inputs["harmonic_frequencies"], np.float32)
    ha = np.ascontiguousarray(np.asarray(inputs["harmonic_amplitudes"], np.float32))
    nba = np.ascontiguousarray(np.asarray(inputs["noisebank_amplitudes"], np.float32))
    nbe = np.asarray(inputs["noisebank_mod_exponents"], np.float32)
    nbw = np.asarray(inputs["noisebank_mod_weights"], np.float32)
    pg = np.asarray(inputs["pulse_noise_gain"], np.float32)
    fg = np.asarray(inputs["flow_noise_gain"], np.float32)
    ip = np.asarray(inputs["initial_phase"], np.float32)
    nbands = np.ascontiguousarray(np.asarray(inputs["noise_bands"], np.float32))

    if "nc" not in _CACHE:
        _CACHE["nc"] = _build()
    nc = _CACHE["nc"]

    # frequencies in turns/sample; antialias mask (never hit for inputs
    # bounded below Nyquist, so only pay for it when needed)
    if float(hf.max()) >= SR / 2:
        hf = hf * (hf < SR / 2)
    hf_t = hf * np.float32(1.0 / SR)
    # first-half phase totals (float64) for the second-half cores
    S = np.sum(hf_t[:, :, :TH], axis=2, dtype=np.float64)
    phi0 = (ip[:, :, 0].astype(np.float64) + np.pi / 2) / (2.0 * np.pi)

    p = np.arange(128)
    m_p = p % 4
    hcb = np.zeros((128, 63), np.float32)
    hcb[:, 31] = 1.0
    nzb = np.zeros((128, 51), np.float32)
    nzb[p, 19 + 4 * m_p] = 1.0
    # mod tile row p = 32h + c
    hm_p = p // 32
    cm_p = p % 32
    # shift matrix: prefix over chunks within each harmonic
    shm = ((hm_p[:, None] == hm_p[None, :]) &
           (cm_p[:, None] < cm_p[None, :])).astype(np.float32)
    # mlh[2g+u][p, o] = w[h]*pg iff c//16 == g and o == 2*(c%16) + u
    mlh_ind = np.zeros((128, 128), np.float32)
    for g in range(2):
        for u in range(2):
            sel = (cm_p // 16 == g)
            mlh_ind[p[sel], 32 * (2 * g + u) + 2 * (cm_p[sel] % 16) + u] = 1.0

    in_maps = []
    for core in range(8):
        b, half = divmod(core, 2)
        ts = slice(half * TH, (half + 1) * TH)
        init = phi0[b] + (S[b] if half == 1 else 0.0)
        init = np.mod(init, 1.0).astype(np.float32).reshape(128, 1)
        initm = init[hm_p]
        mlh = (mlh_ind * (nbw[b, hm_p, 0] * pg[b, 0, 0])[:, None]
               ).astype(np.float32)
        ecol = nbe[b, hm_p, 0].astype(np.float32).reshape(128, 1)
        tgv = np.float32((pg[b, 0, 0] + fg[b, 0, 0]) * 0.7)
        fg3v = np.float32(fg[b, 0, 0] * 0.3)
        in_maps.append(dict(
            f=np.ascontiguousarray(hf_t[b, :, ts]),
            fm=np.ascontiguousarray(hf_t[b, :NM, ts]).reshape(128, TC),
            a=np.ascontiguousarray(ha[b, :, ts]),
            nba=np.ascontiguousarray(
                nba[b, :, ts].reshape(NB, 4, 4, TD).transpose(1, 0, 2, 3)
                .reshape(4, 128, TD)),
            nbb=np.ascontiguousarray(
                nbands[:, ts].reshape(NB, 4, 4, TD).transpose(1, 0, 2, 3)
                .reshape(4, 128, TD)),
            init=init, initm=initm, hcb=hcb, nzb=nzb, mlh=mlh, shm=shm,
            ecol=ecol,
            tg=np.full((128, 1), tgv, np.float32),
            fg3=np.full((128, 1), fg3v, np.float32)))

    _tr = bool(os.environ.get("HNS_TRACE"))
    res = run_bass_kernel_spmd(
        nc, in_maps, core_ids=list(range(8)), trace=_tr,
        tmpdir=os.environ.get("HNS_TRACE_DIR") or None)
    _CACHE["last_res"] = res

    out = np.empty((B, 1, T), np.float32)
    for core in range(8):
        b, half = divmod(core, 2)
        out[b, 0, half * TH:(half + 1) * TH] = \
            res.results[core]["out"].reshape(TH)
    return out
